# revision 16
# baseline (speedup 1.0000x reference)
"""Trainium2 Bass kernel for nn_Block_7584912244953 (gnn_message_passing).

Strategy (8 NeuronCores, SPMD, node-row sharding, no collectives):
  - Associativity: Wp1 @ (e @ W) == (Wp1 @ e) @ W. Each core computes
    T_b^T = e^T @ Wp1_b[rows]^T for its 512 node rows (contraction over
    all 16384 edges), then small (512x512) projections.
  - x_edge (normalized, bf16) is RESIDENT in SBUF (128 KiB/partition),
    loaded once; the four streaming passes then only move the per-core
    Wp1^T (or packed x_node|adj^T) operands in large contiguous DMAs.
  - Four separate branch passes (hh, ee, eh, he) so each branch's SDPA
    (DVE/Pool) overlaps the next branch's PE/DMA stream.
  - Per-node 8x8 attention: big elementwise ops on DVE in bf16 (2x mode,
    both-broadcast access patterns); the two 4096-element reductions are
    split between DVE and the otherwise-idle GPSIMD (Pool) engine;
    softmax normalization applied to the 64-element exp tensor.
  - RMS norms computed on host and folded into the streamed operands;
    gains and 1/sqrt(D) folded into weights. All weights bf16.
"""

import numpy as np
import ml_dtypes

BF16 = ml_dtypes.bfloat16
NCORES = 8
H, D = 8, 64
_CACHE = {}

# SDPA engine-split tuning: of the 64 (h,g) score groups, DVE reduces HS
# and Pool the rest; of the 512 (h,d) output groups, DVE reduces AS.
HS = 20
AS = 160


def _dims(scale=1):
    N, M, E = 4096 // scale, 16384 // scale, 512
    R = N // NCORES
    return dict(N=N, M=M, E=E, R=R, NT=R // 128, ET=E // 128, MT=M // 128,
                NMT=N // 128, F=4 * E, FT=4 * E // 128)


def _build(scale=1, loopn=1, sim_safe=False):
    import concourse.bacc as bacc
    import concourse.mybir as mybir
    from concourse import tile

    dm = _dims(scale)
    N, M, E, R = dm["N"], dm["M"], dm["E"], dm["R"]
    NT, ET, MT, NMT, F, FT = dm["NT"], dm["ET"], dm["MT"], dm["NMT"], dm["F"], dm["FT"]

    F32 = mybir.dt.float32
    F32R = mybir.dt.float32r
    B16 = mybir.dt.bfloat16
    AF = mybir.ActivationFunctionType
    ALU = mybir.AluOpType
    AX = mybir.AxisListType

    nc = bacc.Bacc("TRN2", target_bir_lowering=False, debug=False, num_devices=NCORES)

    # DRAM inputs. All "[128, X]" tensors are host-packed so partition p's
    # data is contiguous (row p holds m-tile-strided rows of the source).
    d_xe = nc.dram_tensor("xe", [128, MT * E], B16, kind="ExternalInput")
    d_hh = nc.dram_tensor("hhpack", [128, NMT * (E + R)], B16, kind="ExternalInput")
    d_w = {b: nc.dram_tensor(f"wst_{b}", [128, MT * R], B16, kind="ExternalInput")
           for b in ("ee", "eh", "he")}
    d_xnt = nc.dram_tensor("xnt", [128, ET * R], B16, kind="ExternalInput")
    PROJ = ["q_hh", "k_hh", "v_hh", "q_ee", "k_ee", "v_ee",
            "q_eh", "k_eh", "v_eh", "q_he", "k_he", "v_he"]
    d_wp = {w: nc.dram_tensor(f"w_{w}", [128, ET * E], B16, kind="ExternalInput")
            for w in PROJ}
    d_wf1 = nc.dram_tensor("wf1", [128, FT * ET * 128], B16, kind="ExternalInput")
    d_wf2 = nc.dram_tensor("wf2", [128, FT * E], B16, kind="ExternalInput")
    d_b1t = nc.dram_tensor("b1t", [128, FT], F32, kind="ExternalInput")
    d_id = nc.dram_tensor("ident", [128, 128], F32, kind="ExternalInput")
    d_ones = nc.dram_tensor("onesrow", [1, 128], B16, kind="ExternalInput")
    d_b2 = nc.dram_tensor("bias2", [1, E], B16, kind="ExternalInput")
    d_out = nc.dram_tensor("out", [R, E], F32, kind="ExternalOutput")

    with tile.TileContext(nc) as tc:
        with (
            tc.tile_pool(name="xeres", bufs=1) as xr,
            tc.tile_pool(name="stream", bufs=2) as st,
            tc.tile_pool(name="wproj", bufs=1) as ws,
            tc.tile_pool(name="tstore", bufs=1) as ts_,
            tc.tile_pool(name="xnts", bufs=1) as xs,
            tc.tile_pool(name="qkv", bufs=1) as qs,
            tc.tile_pool(name="sdpa", bufs=1) as sp,
            tc.tile_pool(name="xacc", bufs=1) as xa,
            tc.tile_pool(name="ffn", bufs=2) as fs,
            tc.tile_pool(name="zst", bufs=1) as zs,
            tc.tile_pool(name="outp", bufs=1) as op_,
            tc.tile_pool(name="psum", bufs=1, space="PSUM") as pp,
            tc.tile_pool(name="misc", bufs=1) as mp,
        ):
            def body(iv=None):
                qkv = {}

                # ---- small residents
                xnt = xs.tile([128, ET * R], B16, tag="xnt", name="xnt")
                nc.sync.dma_start(out=xnt[:], in_=d_xnt.ap())
                ident = mp.tile([128, 128], F32R, tag="ident")
                nc.sync.dma_start(out=ident[:], in_=d_id.ap().bitcast(F32R))
                eps_t = mp.tile([128, 1], F32, tag="eps")
                nc.gpsimd.memset(eps_t[:], 1e-6)
                b1 = mp.tile([128, FT], F32, tag="b1")
                nc.sync.dma_start(out=b1[:], in_=d_b1t.ap())
                ones_t = mp.tile([1, 128], B16, tag="ones")
                nc.sync.dma_start(out=ones_t[:], in_=d_ones.ap())
                b2_t = mp.tile([1, E], B16, tag="b2")
                nc.sync.dma_start(out=b2_t[:], in_=d_b2.ap())

                # ================= pass hh: packed [xnb | adjt] =================
                # 4 m-tiles per DMA (1 MiB); lhsT = xn part, rhs = adj part.
                PB = E + R  # packed row width
                ps_hh = [pp.tile([128, R], F32, tag=f"pbank{e}", name=f"pshh{e}")
                         for e in range(ET)]
                HH_SUB = 2
                for j in range(NMT // HH_SUB):
                    ht = st.tile([128, HH_SUB * PB], B16, tag="hstream")
                    nc.sync.dma_start(
                        out=ht[:], in_=d_hh.ap()[:, j * HH_SUB * PB:(j + 1) * HH_SUB * PB])
                    for s in range(HH_SUB):
                        m = j * HH_SUB + s
                        for e in range(ET):
                            nc.tensor.matmul(
                                ps_hh[e][:],
                                ht[:, s * PB + e * 128:s * PB + (e + 1) * 128],
                                ht[:, s * PB + E:s * PB + PB],
                                start=(m == 0), stop=(m == NMT - 1))
                tstore = {}
                for e in range(ET):
                    tt = ts_.tile([128, R], B16, tag=f"T{e}")
                    nc.scalar.copy(tt[:], ps_hh[e][:])
                    tstore[e] = tt

                # ---- resident x_edge load (overlaps proj/sdpa hh)
                xe_res = xr.tile([128, MT * E], B16, tag="xeres", name="xeres")
                XE_C = 4096  # columns per DMA (1 MiB)
                for j in range((MT * E) // XE_C):
                    nc.sync.dma_start(out=xe_res[:, j * XE_C:(j + 1) * XE_C],
                                      in_=d_xe.ap()[:, j * XE_C:(j + 1) * XE_C])

                # ---- grouped projections
                _wslot = [0]

                def load_w(name):
                    s = _wslot[0] = (_wslot[0] + 1) % 2
                    wt = ws.tile([128, ET * E], B16, tag=f"wp{s}")
                    nc.sync.dma_start(out=wt[:], in_=d_wp[name].ap())
                    return wt

                def proj_group(specs, banks):
                    """specs: list of (name, src_fn). src_fn(k, t) -> lhsT AP
                    [128, 128] (contraction k-slice, output t-slice)."""
                    wts = {name: load_w(name) for name, _ in specs}
                    for t in range(NT):
                        psb = {}
                        for i, (name, srcs) in enumerate(specs):
                            psb[name] = pp.tile([128, E], F32, tag=f"pbank{banks[i]}",
                                                name=f"ps_{name}{t}")
                        for k in range(ET):
                            for name, srcs in specs:
                                nc.tensor.matmul(
                                    psb[name][:], srcs(k, t),
                                    wts[name][:, k * E:(k + 1) * E],
                                    start=(k == 0), stop=(k == ET - 1))
                        for name, srcs in specs:
                            q = qs.tile([128, E], B16, tag=f"{name[0]}{t}")
                            nc.scalar.copy(q[:], psb[name][:])
                            qkv[(name, t)] = q

                def tsrc(k, t):
                    return tstore[k][:, t * 128:(t + 1) * 128]

                def xsrc(k, t):
                    return xnt[:, k * R + t * 128:k * R + (t + 1) * 128]

                # ---- SDPA (DVE + Pool + ACT)
                x_tiles = [xa.tile([128, E], B16, tag=f"x{t}", name=f"x{t}")
                           for t in range(NT)]

                def sdpa(branch, t, first):
                    qb = qkv[(f"q_{branch}", t)]
                    kb = qkv[(f"k_{branch}", t)]
                    vb = qkv[(f"v_{branch}", t)]
                    P = sp.tile([128, H * H * D], B16, tag="P")
                    q_ap = qb[:].rearrange("p (h d) -> p h d", h=H).unsqueeze(2) \
                        .broadcast_to((128, H, H, D))
                    k_ap = kb[:].rearrange("p (g d) -> p g d", g=H).unsqueeze(1) \
                        .broadcast_to((128, H, H, D))
                    nc.vector.tensor_tensor(
                        out=P[:].rearrange("p (h g d) -> p h g d", h=H, g=H),
                        in0=q_ap, in1=k_ap, op=ALU.mult)
                    s_f = sp.tile([128, H * H], F32, tag="s")
                    nc.vector.reduce_sum(
                        out=s_f[:],
                        in_=P[:].rearrange("p (g d) -> p g d", d=D),
                        axis=AX.X)
                    Eb = sp.tile([128, H * H], B16, tag="Eb")
                    nc.scalar.activation(Eb[:], s_f[:], AF.Exp)
                    den = sp.tile([128, H], F32, tag="den")
                    nc.vector.reduce_sum(
                        out=den[:], in_=Eb[:].rearrange("p (h g) -> p h g", g=H),
                        axis=AX.X)
                    rec = sp.tile([128, H], F32, tag="rec")
                    nc.vector.reciprocal(rec[:], den[:])
                    Ebn = sp.tile([128, H * H], B16, tag="Ebn")
                    nc.vector.tensor_tensor(
                        out=Ebn[:].rearrange("p (h g) -> p h g", h=H),
                        in0=Eb[:].rearrange("p (h g) -> p h g", h=H),
                        in1=rec[:].unsqueeze(2).broadcast_to((128, H, H)),
                        op=ALU.mult)
                    Pa = sp.tile([128, H * D * H], B16, tag="P")
                    nc.vector.tensor_tensor(
                        out=Pa[:].rearrange("p (h d g) -> p h d g", h=H, d=D),
                        in0=Ebn[:].rearrange("p (h g) -> p h g", h=H).unsqueeze(2)
                            .broadcast_to((128, H, D, H)),
                        in1=vb[:].rearrange("p (d g) -> p d g", g=H).unsqueeze(1)
                            .broadcast_to((128, H, D, H)),
                        op=ALU.mult)
                    # value-path reduction over g (window 8) as a 3-stage
                    # strided add tree on the Pool engine (scalar_tensor_tensor
                    # with op0=bypass hits the generic-op efficiency class)
                    M8 = H * D  # 512 (h,d) groups
                    Pa3 = Pa[:].rearrange("p (m g) -> p m g", g=H)
                    v1 = sp.tile([128, M8 * 4], B16, tag="vt1")
                    v1_3 = v1[:].rearrange("p (m f) -> p m f", f=4)
                    nc.gpsimd.tensor_tensor(
                        out=v1_3, in0=Pa3[:, :, 0:4], in1=Pa3[:, :, 4:8], op=ALU.add)
                    v2 = sp.tile([128, M8 * 2], B16, tag="vt2")
                    v2_3 = v2[:].rearrange("p (m f) -> p m f", f=2)
                    nc.gpsimd.tensor_tensor(
                        out=v2_3, in0=v1_3[:, :, 0:2], in1=v1_3[:, :, 2:4], op=ALU.add)
                    if first:
                        tgt = x_tiles[t]
                    else:
                        tgt = sp.tile([128, E], B16, tag="av")
                    nc.gpsimd.tensor_tensor(
                        out=tgt[:].rearrange("p (m f) -> p m f", f=1),
                        in0=v2_3[:, :, 0:1], in1=v2_3[:, :, 1:2], op=ALU.add)
                    if not first:
                        nc.vector.tensor_tensor(out=x_tiles[t][:], in0=x_tiles[t][:],
                                                in1=tgt[:], op=ALU.add)

                # ---- hh projections + SDPA; xe keeps loading meanwhile
                proj_group([("k_hh", xsrc), ("v_hh", xsrc)], banks=(0, 1))
                proj_group([("q_hh", tsrc)], banks=(2,))
                for t in range(NT):
                    sdpa("hh", t, first=True)

                # ================= generic streaming pass =================
                def run_pass(branch, pbanks):
                    W_SUB = 2  # m-tiles per DMA (0.25 MiB)
                    ps = [pp.tile([128, R], F32, tag=f"pbank{pbanks[e]}",
                                  name=f"ps{branch}{e}") for e in range(ET)]
                    for j in range(MT // W_SUB):
                        wt = st.tile([128, W_SUB * R], B16, tag="wstream")
                        nc.sync.dma_start(
                            out=wt[:],
                            in_=d_w[branch].ap()[:, j * W_SUB * R:(j + 1) * W_SUB * R])
                        for s in range(W_SUB):
                            m = j * W_SUB + s
                            for e in range(ET):
                                nc.tensor.matmul(
                                    ps[e][:],
                                    xe_res[:, m * E + e * 128:m * E + (e + 1) * 128],
                                    wt[:, s * R:(s + 1) * R],
                                    start=(m == 0), stop=(m == MT - 1))
                    for e in range(ET):
                        tt = ts_.tile([128, R], B16, tag=f"T{e}")
                        nc.scalar.copy(tt[:], ps[e][:])
                        tstore[e] = tt

                # ---- ee
                run_pass("ee", (4, 5, 6, 7))
                proj_group([("q_ee", tsrc), ("k_ee", tsrc)], banks=(4, 5))
                proj_group([("v_ee", xsrc)], banks=(6,))
                for t in range(NT):
                    sdpa("ee", t, first=False)

                # ---- eh
                run_pass("eh", (0, 1, 2, 3))
                proj_group([("q_eh", tsrc)], banks=(0,))
                proj_group([("k_eh", xsrc), ("v_eh", xsrc)], banks=(1, 2))
                for t in range(NT):
                    sdpa("eh", t, first=False)

                # ---- he
                proj_group([("q_he", xsrc), ("v_he", xsrc)], banks=(0, 1))
                run_pass("he", (4, 5, 6, 7))
                proj_group([("k_he", tsrc)], banks=(4,))
                for t in range(NT):
                    sdpa("he", t, first=False)

                # ================= FFN =================
                yT = xa.tile([128, ET * R], B16, tag="yT", name="yT")
                for t in range(NT):
                    xt = x_tiles[t]
                    scr = sp.tile([128, E], F32, tag="ffnscr")
                    nc.scalar.activation(scr[:], xt[:], AF.Square)
                    ms = mp.tile([128, 1], F32, tag=f"ms{t}")
                    nc.vector.reduce_sum(out=ms[:], in_=scr[:], axis=AX.X)
                    sd = mp.tile([128, 1], F32, tag=f"sd{t}")
                    nc.scalar.activation(sd[:], ms[:], AF.Sqrt, scale=1.0 / E,
                                         bias=eps_t[:])
                    inv2 = mp.tile([128, 1], F32, tag=f"inv{t}")
                    nc.vector.reciprocal(inv2[:], sd[:])
                    yt = sp.tile([128, E], F32R, tag="ffny")
                    nc.vector.tensor_scalar_mul(yt[:], xt[:], inv2[:])
                    for e in range(ET):
                        pst = pp.tile([128, 128], F32R, tag=f"pbank{e % 2}")
                        nc.tensor.transpose(pst[:], yt[:, e * 128:(e + 1) * 128],
                                            ident[:])
                        nc.scalar.copy(yT[:, e * R + t * 128:e * R + (t + 1) * 128],
                                       pst[:])
                pso = [pp.tile([128, E], F32, tag=f"pbank{4 + t}", name=f"pso{t}")
                       for t in range(NT)]
                WF_F = 1  # f-blocks per wf DMA
                for fj in range(FT // WF_F):
                    w1 = fs.tile([128, WF_F * ET * 128], B16, tag="wf1s")
                    nc.sync.dma_start(
                        out=w1[:],
                        in_=d_wf1.ap()[:, fj * WF_F * E:(fj + 1) * WF_F * E])
                    w2 = fs.tile([128, WF_F * E], B16, tag="wf2s")
                    nc.sync.dma_start(
                        out=w2[:],
                        in_=d_wf2.ap()[:, fj * WF_F * E:(fj + 1) * WF_F * E])
                    for s in range(WF_F):
                        f = fj * WF_F + s
                        psz = pp.tile([128, R], F32, tag=f"pbank{2 + (f % 2)}")
                        for k in range(ET):
                            nc.tensor.matmul(
                                psz[:], w1[:, (s * ET + k) * 128:(s * ET + k + 1) * 128],
                                yT[:, k * R:(k + 1) * R],
                                start=(k == 0), stop=(k == ET - 1))
                        zt = zs.tile([128, R], B16, tag=f"zT{f % 2}")
                        nc.scalar.activation(zt[:], psz[:],
                                             AF.Identity if sim_safe else AF.Gelu,
                                             bias=b1[:, f:f + 1])
                        for t in range(NT):
                            nc.tensor.matmul(pso[t][:], zt[:, t * 128:(t + 1) * 128],
                                             w2[:, s * E:(s + 1) * E],
                                             start=(f == 0), stop=False)
                for t in range(NT):
                    nc.tensor.matmul(pso[t][:], ones_t[0:1, :], b2_t[0:1, :],
                                     start=False, stop=True)
                    ot = op_.tile([128, E], F32, tag="ot")
                    nc.scalar.copy(ot[:], pso[t][:])
                    nc.sync.dma_start(out=d_out.ap()[t * 128:(t + 1) * 128, :],
                                      in_=ot[:])

            if loopn > 1:
                with tc.For_i(0, loopn, 1) as _i:
                    body(_i)
            else:
                body()

    nc.compile()
    return nc


def _prep_inputs(inputs, scale=1):
    """Host-side folding + packing + sharding. Returns per-core in_maps."""
    dm = _dims(scale)
    N, M, E, R, F, FT = dm["N"], dm["M"], dm["E"], dm["R"], dm["F"], dm["FT"]
    ET, MT, NMT = dm["ET"], dm["MT"], dm["NMT"]
    x_node = np.asarray(inputs["x_node"], np.float32)
    x_edge = np.asarray(inputs["x_edge"], np.float32)
    adj = np.asarray(inputs["adj"], np.float32)
    g_n = np.asarray(inputs["g_n"], np.float32)
    g_e = np.asarray(inputs["g_e"], np.float32)
    g2 = np.asarray(inputs["g2"], np.float32)

    inv_n = (1.0 / np.sqrt((x_node.astype(np.float64) ** 2).mean(axis=1) + 1e-6)).astype(np.float32)
    inv_e = (1.0 / np.sqrt((x_edge.astype(np.float64) ** 2).mean(axis=1) + 1e-6)).astype(np.float32)
    xn_s = x_node * inv_n[:, None]
    xe_s = x_edge * inv_e[:, None]

    perm = np.array([(j % H) * D + j // H for j in range(E)])  # v cols -> d-major

    def fold_q(w, g):
        return (g[:, None] * np.asarray(w, np.float32)) / np.sqrt(D)

    def fold_k(w, g):
        return g[:, None] * np.asarray(w, np.float32)

    def fold_v(w, g):
        return (g[:, None] * np.asarray(w, np.float32))[:, perm]

    wp = {
        "q_hh": fold_q(inputs["Wq_hh"], g_n),
        "k_hh": fold_k(inputs["Wk_hh"], g_n),
        "v_hh": fold_v(inputs["Wv_hh"], g_n),
        "q_ee": fold_q(inputs["Wq_ee"], g_e),
        "k_ee": fold_k(inputs["Wk_ee"], g_e),
        "v_ee": fold_v(inputs["Wv_ee"], g_n),
        "q_eh": fold_q(inputs["Wq_eh"], g_e),
        "k_eh": fold_k(inputs["Wk_eh"], g_n),
        "v_eh": fold_v(inputs["Wv_eh"], g_n),
        "q_he": fold_q(inputs["Wq_he"], g_n),
        "k_he": fold_k(inputs["Wk_he"], g_e),
        "v_he": fold_v(inputs["Wv_he"], g_n),
    }

    def mpack(a):
        # [rows, C] -> [128, (rows/128) * C]; partition p block j = a[j*128+p]
        rows, C = a.shape
        return np.ascontiguousarray(
            a.reshape(rows // 128, 128, C).transpose(1, 0, 2).reshape(128, -1))

    wf1 = (g2[:, None] * np.asarray(inputs["Wf1"], np.float32))  # [E, F]
    # wf1 pack: partition p, col-block (f*ET+k) = wf1[k*128+p, f*128:(f+1)*128]
    wf1p = wf1.reshape(ET, 128, FT, 128).transpose(1, 2, 0, 3).reshape(128, -1)
    wf2 = np.asarray(inputs["Wf2"], np.float32)  # [F, E]
    wf2p = mpack(wf2)
    bf1 = np.asarray(inputs["bf1"], np.float32)
    b1t = np.ascontiguousarray(bf1.reshape(FT, 128).T)

    shared = {
        "xe": mpack(xe_s).astype(BF16),
        "b1t": b1t,
        "wf1": np.ascontiguousarray(wf1p).astype(BF16),
        "wf2": wf2p.astype(BF16),
        "ident": np.eye(128, dtype=np.float32),
        "onesrow": np.ones((1, 128), BF16),
        "bias2": np.asarray(inputs["bf2"], np.float32)[None, :].astype(BF16),
    }
    for k, v in wp.items():
        # [E, E] -> [128, ET*E]: partition p block k = W[k*128+p, :]
        shared[f"w_{k}"] = mpack(v).astype(BF16)

    wp1 = {b: np.asarray(inputs[f"Wp1_{b}"], np.float32) for b in ("ee", "eh", "he")}
    xnb = xn_s.astype(BF16)
    in_maps = []
    for c in range(NCORES):
        rows = slice(c * R, (c + 1) * R)
        m = dict(shared)
        # hhpack: per m-tile j: [xnb rows | adjt rows] where adjt = adj[rows].T
        adjt = np.ascontiguousarray(adj[rows].T).astype(BF16)  # [N, R]
        hh = np.concatenate([xnb.reshape(NMT, 128, E),
                             adjt.reshape(NMT, 128, R)], axis=2)  # [NMT,128,E+R]
        m["hhpack"] = np.ascontiguousarray(hh.transpose(1, 0, 2).reshape(128, -1))
        m["xnt"] = mpack(np.ascontiguousarray(xn_s[rows].T)).astype(BF16)
        for b in ("ee", "eh", "he"):
            m[f"wst_{b}"] = mpack(np.ascontiguousarray(wp1[b][rows].T)).astype(BF16)
        in_maps.append(m)
    return in_maps


def kernel(**inputs) -> np.ndarray:
    from concourse.bass_utils import run_bass_kernel_spmd

    if "nc" not in _CACHE:
        _CACHE["nc"] = _build()
    nc = _CACHE["nc"]
    in_maps = _prep_inputs(inputs)
    res = run_bass_kernel_spmd(nc, in_maps, list(range(NCORES)))
    out = np.concatenate([res.results[c]["out"] for c in range(NCORES)], axis=0)
    return np.ascontiguousarray(out, dtype=np.float32)


# revision 24
# speedup vs baseline: 1.2384x; 1.2384x over previous
"""Trainium2 Bass kernel for nn_Block_7584912244953 (gnn_message_passing).

Strategy (8 NeuronCores, SPMD, node-row sharding, no collectives):
  - Associativity: Wp1 @ (e @ W) == (Wp1 @ e) @ W. Each core computes
    T_b^T = e^T @ Wp1_b[rows]^T for its 512 node rows (contraction over
    all 16384 edges), then small (512x512) projections.
  - x_edge (normalized, bf16) is RESIDENT in SBUF (128 KiB/partition),
    loaded once; the four streaming passes then only move the per-core
    Wp1^T (or packed x_node|adj^T) operands in large contiguous DMAs.
  - Four separate branch passes (hh, ee, eh, he) so each branch's SDPA
    (DVE/Pool) overlaps the next branch's PE/DMA stream.
  - Per-node 8x8 attention: big elementwise ops on DVE in bf16 (2x mode,
    both-broadcast access patterns); the two 4096-element reductions are
    split between DVE and the otherwise-idle GPSIMD (Pool) engine;
    softmax normalization applied to the 64-element exp tensor.
  - RMS norms computed on host and folded into the streamed operands;
    gains and 1/sqrt(D) folded into weights. All weights bf16.
"""

import numpy as np
import ml_dtypes

BF16 = ml_dtypes.bfloat16
NCORES = 8
H, D = 8, 64
_CACHE = {}

# SDPA engine-split tuning: of the 64 (h,g) score groups, DVE reduces HS
# and Pool the rest; of the 512 (h,d) output groups, DVE reduces AS.
HS = 20
AS = 160


def _dims(scale=1):
    N, M, E = 4096 // scale, 16384 // scale, 512
    R = N // NCORES
    return dict(N=N, M=M, E=E, R=R, NT=R // 128, ET=E // 128, MT=M // 128,
                NMT=N // 128, F=4 * E, FT=4 * E // 128)


def _build(scale=1, loopn=1, sim_safe=False):
    import os
    import concourse.bacc as bacc
    import concourse.mybir as mybir
    from concourse import tile

    ablate = os.environ.get("ABLATE", "")

    dm = _dims(scale)
    N, M, E, R = dm["N"], dm["M"], dm["E"], dm["R"]
    NT, ET, MT, NMT, F, FT = dm["NT"], dm["ET"], dm["MT"], dm["NMT"], dm["F"], dm["FT"]

    F32 = mybir.dt.float32
    F32R = mybir.dt.float32r
    B16 = mybir.dt.bfloat16
    AF = mybir.ActivationFunctionType
    ALU = mybir.AluOpType
    AX = mybir.AxisListType

    nc = bacc.Bacc("TRN2", target_bir_lowering=False, debug=False, num_devices=NCORES)

    # DRAM inputs. All "[128, X]" tensors are host-packed so partition p's
    # data is contiguous (row p holds m-tile-strided rows of the source).
    d_xe = nc.dram_tensor("xe", [128, MT * E], B16, kind="ExternalInput")
    d_hh = nc.dram_tensor("hhpack", [128, NMT * (E + R)], B16, kind="ExternalInput")
    d_w = {b: nc.dram_tensor(f"wst_{b}", [128, MT * R], B16, kind="ExternalInput")
           for b in ("ee", "eh", "he")}
    d_xnt = nc.dram_tensor("xnt", [128, ET * R], B16, kind="ExternalInput")
    PROJ = ["q_hh", "k_hh", "v_hh", "q_ee", "k_ee", "v_ee",
            "q_eh", "k_eh", "v_eh", "q_he", "k_he", "v_he"]
    d_wp = {w: nc.dram_tensor(f"w_{w}", [128, ET * E], B16, kind="ExternalInput")
            for w in PROJ}
    d_wf1 = nc.dram_tensor("wf1", [128, FT * ET * 128], B16, kind="ExternalInput")
    d_wf2 = nc.dram_tensor("wf2", [128, FT * E], B16, kind="ExternalInput")
    d_b1t = nc.dram_tensor("b1t", [128, FT], F32, kind="ExternalInput")
    d_id = nc.dram_tensor("ident", [128, 128], B16, kind="ExternalInput")
    d_ones = nc.dram_tensor("onesrow", [1, 128], B16, kind="ExternalInput")
    d_b2 = nc.dram_tensor("bias2", [1, E], B16, kind="ExternalInput")
    d_out = nc.dram_tensor("out", [R, E], F32, kind="ExternalOutput")

    with tile.TileContext(nc) as tc:
        with (
            tc.tile_pool(name="xeres", bufs=1) as xr,
            tc.tile_pool(name="stream", bufs=4) as st,
            tc.tile_pool(name="wproj", bufs=1) as ws,
            tc.tile_pool(name="tstore", bufs=1) as ts_,
            tc.tile_pool(name="xnts", bufs=1) as xs,
            tc.tile_pool(name="qkv", bufs=1) as qs,
            tc.tile_pool(name="sdpa", bufs=1) as sp,
            tc.tile_pool(name="xacc", bufs=1) as xa,
            tc.tile_pool(name="ffn", bufs=2) as fs,
            tc.tile_pool(name="zst", bufs=1) as zs,
            tc.tile_pool(name="outp", bufs=1) as op_,
            tc.tile_pool(name="psum", bufs=1, space="PSUM") as pp,
            tc.tile_pool(name="misc", bufs=1) as mp,
        ):
            def body(iv=None):
                qkv = {}

                # ---- small residents
                xnt = xs.tile([128, ET * R], B16, tag="xnt", name="xnt")
                nc.sync.dma_start(out=xnt[:], in_=d_xnt.ap())
                ident = mp.tile([128, 128], B16, tag="ident")
                nc.sync.dma_start(out=ident[:], in_=d_id.ap())
                eps_t = mp.tile([128, 1], F32, tag="eps")
                nc.gpsimd.memset(eps_t[:], 1e-6)
                b1 = mp.tile([128, FT], F32, tag="b1")
                nc.sync.dma_start(out=b1[:], in_=d_b1t.ap())
                ones_t = mp.tile([1, 128], B16, tag="ones")
                nc.sync.dma_start(out=ones_t[:], in_=d_ones.ap())
                b2_t = mp.tile([1, E], B16, tag="b2")
                nc.sync.dma_start(out=b2_t[:], in_=d_b2.ap())

                xe_res = xr.tile([128, MT * E], B16, tag="xeres", name="xeres")
                tstore = {}

                # ---- grouped projections
                _wslot = [0]

                def load_w(name):
                    s = _wslot[0] = (_wslot[0] + 1) % 2
                    wt = ws.tile([128, ET * E], B16, tag=f"wp{s}")
                    nc.sync.dma_start(out=wt[:], in_=d_wp[name].ap())
                    return wt

                def proj_group(specs, banks):
                    """specs: list of (name, src_fn). src_fn(k, t) -> lhsT AP
                    [128, 128] (contraction k-slice, output t-slice)."""
                    wts = {name: load_w(name) for name, _ in specs}
                    for t in range(NT):
                        psb = {}
                        for i, (name, srcs) in enumerate(specs):
                            psb[name] = pp.tile([128, E], F32, tag=f"pbank{banks[i]}",
                                                name=f"ps_{name}{t}")
                        for k in range(ET):
                            for name, srcs in specs:
                                nc.tensor.matmul(
                                    psb[name][:], srcs(k, t),
                                    wts[name][:, k * E:(k + 1) * E],
                                    start=(k == 0), stop=(k == ET - 1))
                        for name, srcs in specs:
                            q = qs.tile([128, E], B16, tag=f"{name[0]}{t}")
                            nc.scalar.copy(q[:], psb[name][:])
                            qkv[(name, t)] = q

                def tsrc(k, t):
                    return tstore[k][:, t * 128:(t + 1) * 128]

                def xsrc(k, t):
                    return xnt[:, k * R + t * 128:k * R + (t + 1) * 128]

                # ---- SDPA (DVE + Pool + ACT)
                x_tiles = [xa.tile([128, E], B16, tag=f"x{t}", name=f"x{t}")
                           for t in range(NT)]
                if ablate == "nosdpa":
                    for t in range(NT):
                        nc.gpsimd.memset(x_tiles[t][:], 0.25)

                def sdpa(branch, t, first):
                    if ablate == "nosdpa":
                        return
                    tree_eng = nc.gpsimd if ablate == "pool" else nc.vector
                    qb = qkv[(f"q_{branch}", t)]
                    kb = qkv[(f"k_{branch}", t)]
                    vb = qkv[(f"v_{branch}", t)]
                    P = sp.tile([128, H * H * D], B16, tag="P")
                    q_ap = qb[:].rearrange("p (h d) -> p h d", h=H).unsqueeze(2) \
                        .broadcast_to((128, H, H, D))
                    k_ap = kb[:].rearrange("p (g d) -> p g d", g=H).unsqueeze(1) \
                        .broadcast_to((128, H, H, D))
                    nc.vector.tensor_tensor(
                        out=P[:].rearrange("p (h g d) -> p h g d", h=H, g=H),
                        in0=q_ap, in1=k_ap, op=ALU.mult)
                    P3 = P[:].rearrange("p (g d) -> p g d", d=D)
                    s2 = sp.tile([128, H * H * D // 2], B16, tag="vt1")
                    s2_3 = s2[:].rearrange("p (g d) -> p g d", d=D // 2)
                    nc.vector.tensor_tensor(
                        out=s2_3, in0=P3[:, :, 0:D // 2], in1=P3[:, :, D // 2:D],
                        op=ALU.add)
                    s_f = sp.tile([128, H * H], F32, tag="s")
                    nc.vector.reduce_sum(out=s_f[:], in_=s2_3, axis=AX.X)
                    Eb = sp.tile([128, H * H], B16, tag="Eb")
                    nc.scalar.activation(Eb[:], s_f[:], AF.Exp)
                    den = sp.tile([128, H], F32, tag="den")
                    nc.vector.reduce_sum(
                        out=den[:], in_=Eb[:].rearrange("p (h g) -> p h g", g=H),
                        axis=AX.X)
                    rec = sp.tile([128, H], F32, tag="rec")
                    nc.vector.reciprocal(rec[:], den[:])
                    Ebn = sp.tile([128, H * H], B16, tag="Ebn")
                    nc.vector.tensor_tensor(
                        out=Ebn[:].rearrange("p (h g) -> p h g", h=H),
                        in0=Eb[:].rearrange("p (h g) -> p h g", h=H),
                        in1=rec[:].unsqueeze(2).broadcast_to((128, H, H)),
                        op=ALU.mult)
                    Pa = sp.tile([128, H * D * H], B16, tag="P")
                    nc.vector.tensor_tensor(
                        out=Pa[:].rearrange("p (h d g) -> p h d g", h=H, d=D),
                        in0=Ebn[:].rearrange("p (h g) -> p h g", h=H).unsqueeze(2)
                            .broadcast_to((128, H, D, H)),
                        in1=vb[:].rearrange("p (d g) -> p d g", g=H).unsqueeze(1)
                            .broadcast_to((128, H, D, H)),
                        op=ALU.mult)
                    # value-path reduction over g (window 8) as a 3-stage
                    # strided add tree on the Pool engine (scalar_tensor_tensor
                    # with op0=bypass hits the generic-op efficiency class)
                    M8 = H * D  # 512 (h,d) groups
                    Pa3 = Pa[:].rearrange("p (m g) -> p m g", g=H)
                    v1 = sp.tile([128, M8 * 4], B16, tag="vt1")
                    v1_3 = v1[:].rearrange("p (m f) -> p m f", f=4)
                    tree_eng.tensor_tensor(
                        out=v1_3, in0=Pa3[:, :, 0:4], in1=Pa3[:, :, 4:8], op=ALU.add)
                    v2 = sp.tile([128, M8 * 2], B16, tag="vt2")
                    v2_3 = v2[:].rearrange("p (m f) -> p m f", f=2)
                    tree_eng.tensor_tensor(
                        out=v2_3, in0=v1_3[:, :, 0:2], in1=v1_3[:, :, 2:4], op=ALU.add)
                    if first:
                        tgt = x_tiles[t]
                    else:
                        tgt = sp.tile([128, E], B16, tag="av")
                    tree_eng.tensor_tensor(
                        out=tgt[:].rearrange("p (m f) -> p m f", f=1),
                        in0=v2_3[:, :, 0:1], in1=v2_3[:, :, 1:2], op=ALU.add)
                    if not first:
                        nc.vector.tensor_tensor(out=x_tiles[t][:], in0=x_tiles[t][:],
                                                in1=tgt[:], op=ALU.add)

                # ---- hh: early node-side projections fill PE during the
                # first stream DMAs, then the packed [xnb | adjt] pass.
                proj_group([("k_hh", xsrc), ("v_hh", xsrc)], banks=(4, 5))
                PB = E + R  # packed row width
                ps_hh = [pp.tile([128, R], F32, tag=f"pbank{e}", name=f"pshh{e}")
                         for e in range(ET)]
                HH_SUB = 2
                for j in range(NMT // HH_SUB):
                    ht = st.tile([128, HH_SUB * PB], B16, tag="stream")
                    nc.sync.dma_start(
                        out=ht[:], in_=d_hh.ap()[:, j * HH_SUB * PB:(j + 1) * HH_SUB * PB])
                    for s in range(HH_SUB):
                        m = j * HH_SUB + s
                        for e in range(ET):
                            nc.tensor.matmul(
                                ps_hh[e][:],
                                ht[:, s * PB + e * 128:s * PB + (e + 1) * 128],
                                ht[:, s * PB + E:s * PB + PB],
                                start=(m == 0), stop=(m == NMT - 1))
                for e in range(ET):
                    tt = ts_.tile([128, R], B16, tag=f"T{e}")
                    nc.scalar.copy(tt[:], ps_hh[e][:])
                    tstore[e] = tt
                proj_group([("q_hh", tsrc)], banks=(2,))
                for t in range(NT):
                    sdpa("hh", t, first=True)

                # ================= generic streaming pass =================
                XE_C = 4096  # xe columns (elements) per interleaved DMA (1 MiB)

                def xe_dma(b):
                    nc.sync.dma_start(
                        out=xe_res[:, b * XE_C:(b + 1) * XE_C],
                        in_=d_xe.ap()[:, b * XE_C:(b + 1) * XE_C])

                def run_pass(branch, pbanks, xe_load=False):
                    W_SUB = 4  # m-tiles per DMA (0.5 MiB)
                    ps = [pp.tile([128, R], F32, tag=f"pbank{pbanks[e]}",
                                  name=f"ps{branch}{e}") for e in range(ET)]
                    if xe_load:
                        xe_dma(0)
                        xe_dma(1)
                    for j in range(MT // W_SUB):
                        # interleave the resident-xe load two blocks ahead of
                        # its first consumer m-tile
                        if xe_load and j % 2 == 0 and j // 2 + 2 < (MT * E) // XE_C:
                            xe_dma(j // 2 + 2)
                        wt = st.tile([128, W_SUB * R], B16, tag="stream")
                        nc.sync.dma_start(
                            out=wt[:],
                            in_=d_w[branch].ap()[:, j * W_SUB * R:(j + 1) * W_SUB * R])
                        for s in range(W_SUB):
                            m = j * W_SUB + s
                            for e in range(ET):
                                nc.tensor.matmul(
                                    ps[e][:],
                                    xe_res[:, m * E + e * 128:m * E + (e + 1) * 128],
                                    wt[:, s * R:(s + 1) * R],
                                    start=(m == 0), stop=(m == MT - 1))
                    for e in range(ET):
                        tt = ts_.tile([128, R], B16, tag=f"T{e}")
                        nc.scalar.copy(tt[:], ps[e][:])
                        tstore[e] = tt

                # ---- ee
                run_pass("ee", (4, 5, 6, 7), xe_load=True)
                proj_group([("q_ee", tsrc), ("k_ee", tsrc)], banks=(4, 5))
                proj_group([("v_ee", xsrc)], banks=(6,))
                for t in range(NT):
                    sdpa("ee", t, first=False)

                # ---- eh
                run_pass("eh", (0, 1, 2, 3))
                proj_group([("q_eh", tsrc)], banks=(0,))
                proj_group([("k_eh", xsrc), ("v_eh", xsrc)], banks=(1, 2))
                for t in range(NT):
                    sdpa("eh", t, first=False)

                # ---- he
                proj_group([("q_he", xsrc), ("v_he", xsrc)], banks=(0, 1))
                run_pass("he", (4, 5, 6, 7))
                proj_group([("k_he", tsrc)], banks=(4,))

                # ================= FFN (two node-halves so half A overlaps
                # the tail SDPA of tiles 2,3) =================
                yT = xa.tile([128, ET * R], B16, tag="yT", name="yT")

                def rms_transpose(t):
                    xt = x_tiles[t]
                    scr = sp.tile([128, E], B16, tag="ffnscr")
                    nc.scalar.activation(scr[:], xt[:], AF.Square)
                    ms = mp.tile([128, 1], F32, tag=f"ms{t}")
                    nc.vector.reduce_sum(out=ms[:], in_=scr[:], axis=AX.X)
                    sd = mp.tile([128, 1], F32, tag=f"sd{t}")
                    nc.scalar.activation(sd[:], ms[:], AF.Sqrt, scale=1.0 / E,
                                         bias=eps_t[:])
                    inv2 = mp.tile([128, 1], F32, tag=f"inv{t}")
                    nc.vector.reciprocal(inv2[:], sd[:])
                    yt = sp.tile([128, E], B16, tag="ffny")
                    nc.vector.tensor_scalar_mul(yt[:], xt[:], inv2[:])
                    for e in range(ET):
                        pst = pp.tile([128, 128], B16, tag=f"pbank{e % 2}")
                        nc.tensor.transpose(pst[:], yt[:, e * 128:(e + 1) * 128],
                                            ident[:])
                        nc.scalar.copy(yT[:, e * R + t * 128:e * R + (t + 1) * 128],
                                       pst[:])

                def ffn_half(h):
                    RH = R // 2  # node columns per half
                    c0 = h * RH
                    pso = [pp.tile([128, E], F32, tag=f"pbank{5 + t}",
                                   name=f"pso{h}{t}") for t in range(2)]
                    for f in range(FT):
                        w1 = fs.tile([128, ET * 128], B16, tag="wf1s")
                        nc.sync.dma_start(
                            out=w1[:], in_=d_wf1.ap()[:, f * E:(f + 1) * E])
                        w2 = fs.tile([128, E], B16, tag="wf2s")
                        nc.sync.dma_start(
                            out=w2[:], in_=d_wf2.ap()[:, f * E:(f + 1) * E])
                        psz = pp.tile([128, RH], F32, tag=f"pbank{2 + (f % 2)}")
                        for k in range(ET):
                            nc.tensor.matmul(
                                psz[:], w1[:, k * 128:(k + 1) * 128],
                                yT[:, k * R + c0:k * R + c0 + RH],
                                start=(k == 0), stop=(k == ET - 1))
                        zt = zs.tile([128, RH], B16, tag=f"zT{f % 2}")
                        nc.scalar.activation(zt[:], psz[:],
                                             AF.Identity if sim_safe else AF.Gelu,
                                             bias=b1[:, f:f + 1])
                        for t in range(2):
                            nc.tensor.matmul(pso[t][:], zt[:, t * 128:(t + 1) * 128],
                                             w2[:],
                                             start=(f == 0), stop=False)
                    for t in range(2):
                        nc.tensor.matmul(pso[t][:], ones_t[0:1, :], b2_t[0:1, :],
                                         start=False, stop=True)
                        ot = op_.tile([128, E], F32, tag="ot")
                        nc.scalar.copy(ot[:], pso[t][:])
                        r0 = (2 * h + t) * 128
                        nc.sync.dma_start(out=d_out.ap()[r0:r0 + 128, :], in_=ot[:])

                sdpa("he", 0, first=False)
                sdpa("he", 1, first=False)
                rms_transpose(0)
                rms_transpose(1)
                sdpa("he", 2, first=False)
                sdpa("he", 3, first=False)
                ffn_half(0)
                rms_transpose(2)
                rms_transpose(3)
                ffn_half(1)

            if loopn > 1:
                with tc.For_i(0, loopn, 1) as _i:
                    body(_i)
            else:
                body()

    nc.compile()
    return nc


def _prep_inputs(inputs, scale=1):
    """Host-side folding + packing + sharding. Returns per-core in_maps."""
    dm = _dims(scale)
    N, M, E, R, F, FT = dm["N"], dm["M"], dm["E"], dm["R"], dm["F"], dm["FT"]
    ET, MT, NMT = dm["ET"], dm["MT"], dm["NMT"]
    x_node = np.asarray(inputs["x_node"], np.float32)
    x_edge = np.asarray(inputs["x_edge"], np.float32)
    adj = np.asarray(inputs["adj"], np.float32)
    g_n = np.asarray(inputs["g_n"], np.float32)
    g_e = np.asarray(inputs["g_e"], np.float32)
    g2 = np.asarray(inputs["g2"], np.float32)

    inv_n = (1.0 / np.sqrt((x_node.astype(np.float64) ** 2).mean(axis=1) + 1e-6)).astype(np.float32)
    inv_e = (1.0 / np.sqrt((x_edge.astype(np.float64) ** 2).mean(axis=1) + 1e-6)).astype(np.float32)
    xn_s = x_node * inv_n[:, None]
    xe_s = x_edge * inv_e[:, None]

    perm = np.array([(j % H) * D + j // H for j in range(E)])  # v cols -> d-major

    def fold_q(w, g):
        return (g[:, None] * np.asarray(w, np.float32)) / np.sqrt(D)

    def fold_k(w, g):
        return g[:, None] * np.asarray(w, np.float32)

    def fold_v(w, g):
        return (g[:, None] * np.asarray(w, np.float32))[:, perm]

    wp = {
        "q_hh": fold_q(inputs["Wq_hh"], g_n),
        "k_hh": fold_k(inputs["Wk_hh"], g_n),
        "v_hh": fold_v(inputs["Wv_hh"], g_n),
        "q_ee": fold_q(inputs["Wq_ee"], g_e),
        "k_ee": fold_k(inputs["Wk_ee"], g_e),
        "v_ee": fold_v(inputs["Wv_ee"], g_n),
        "q_eh": fold_q(inputs["Wq_eh"], g_e),
        "k_eh": fold_k(inputs["Wk_eh"], g_n),
        "v_eh": fold_v(inputs["Wv_eh"], g_n),
        "q_he": fold_q(inputs["Wq_he"], g_n),
        "k_he": fold_k(inputs["Wk_he"], g_e),
        "v_he": fold_v(inputs["Wv_he"], g_n),
    }

    def mpack(a):
        # [rows, C] -> [128, (rows/128) * C]; partition p block j = a[j*128+p]
        rows, C = a.shape
        return np.ascontiguousarray(
            a.reshape(rows // 128, 128, C).transpose(1, 0, 2).reshape(128, -1))

    wf1 = (g2[:, None] * np.asarray(inputs["Wf1"], np.float32))  # [E, F]
    # wf1 pack: partition p, col-block (f*ET+k) = wf1[k*128+p, f*128:(f+1)*128]
    wf1p = wf1.reshape(ET, 128, FT, 128).transpose(1, 2, 0, 3).reshape(128, -1)
    wf2 = np.asarray(inputs["Wf2"], np.float32)  # [F, E]
    wf2p = mpack(wf2)
    bf1 = np.asarray(inputs["bf1"], np.float32)
    b1t = np.ascontiguousarray(bf1.reshape(FT, 128).T)

    shared = {
        "xe": mpack(xe_s).astype(BF16),
        "b1t": b1t,
        "wf1": np.ascontiguousarray(wf1p).astype(BF16),
        "wf2": wf2p.astype(BF16),
        "ident": np.eye(128, dtype=np.float32).astype(BF16),
        "onesrow": np.ones((1, 128), BF16),
        "bias2": np.asarray(inputs["bf2"], np.float32)[None, :].astype(BF16),
    }
    for k, v in wp.items():
        # [E, E] -> [128, ET*E]: partition p block k = W[k*128+p, :]
        shared[f"w_{k}"] = mpack(v).astype(BF16)

    wp1 = {b: np.asarray(inputs[f"Wp1_{b}"], np.float32) for b in ("ee", "eh", "he")}
    xnb = xn_s.astype(BF16)
    in_maps = []
    for c in range(NCORES):
        rows = slice(c * R, (c + 1) * R)
        m = dict(shared)
        # hhpack: per m-tile j: [xnb rows | adjt rows] where adjt = adj[rows].T
        adjt = np.ascontiguousarray(adj[rows].T).astype(BF16)  # [N, R]
        hh = np.concatenate([xnb.reshape(NMT, 128, E),
                             adjt.reshape(NMT, 128, R)], axis=2)  # [NMT,128,E+R]
        m["hhpack"] = np.ascontiguousarray(hh.transpose(1, 0, 2).reshape(128, -1))
        m["xnt"] = mpack(np.ascontiguousarray(xn_s[rows].T)).astype(BF16)
        for b in ("ee", "eh", "he"):
            m[f"wst_{b}"] = mpack(np.ascontiguousarray(wp1[b][rows].T)).astype(BF16)
        in_maps.append(m)
    return in_maps


def kernel(**inputs) -> np.ndarray:
    from concourse.bass_utils import run_bass_kernel_spmd

    if "nc" not in _CACHE:
        _CACHE["nc"] = _build()
    nc = _CACHE["nc"]
    in_maps = _prep_inputs(inputs)
    res = run_bass_kernel_spmd(nc, in_maps, list(range(NCORES)))
    out = np.concatenate([res.results[c]["out"] for c in range(NCORES)], axis=0)
    return np.ascontiguousarray(out, dtype=np.float32)


# revision 25
# speedup vs baseline: 1.2523x; 1.0113x over previous
"""Trainium2 Bass kernel for nn_Block_7584912244953 (gnn_message_passing).

Strategy (8 NeuronCores, SPMD, node-row sharding, no collectives):
  - Associativity: Wp1 @ (e @ W) == (Wp1 @ e) @ W. Each core computes
    T_b^T = e^T @ Wp1_b[rows]^T for its 512 node rows (contraction over
    all 16384 edges), then small (512x512) projections.
  - x_edge (normalized, bf16) is RESIDENT in SBUF (128 KiB/partition),
    loaded once; the four streaming passes then only move the per-core
    Wp1^T (or packed x_node|adj^T) operands in large contiguous DMAs.
  - Four separate branch passes (hh, ee, eh, he) so each branch's SDPA
    (DVE/Pool) overlaps the next branch's PE/DMA stream.
  - Per-node 8x8 attention: big elementwise ops on DVE in bf16 (2x mode,
    both-broadcast access patterns); the two 4096-element reductions are
    split between DVE and the otherwise-idle GPSIMD (Pool) engine;
    softmax normalization applied to the 64-element exp tensor.
  - RMS norms computed on host and folded into the streamed operands;
    gains and 1/sqrt(D) folded into weights. All weights bf16.
"""

import numpy as np
import ml_dtypes

BF16 = ml_dtypes.bfloat16
NCORES = 8
H, D = 8, 64
_CACHE = {}

# SDPA engine-split tuning: of the 64 (h,g) score groups, DVE reduces HS
# and Pool the rest; of the 512 (h,d) output groups, DVE reduces AS.
HS = 20
AS = 160


def _dims(scale=1):
    N, M, E = 4096 // scale, 16384 // scale, 512
    R = N // NCORES
    return dict(N=N, M=M, E=E, R=R, NT=R // 128, ET=E // 128, MT=M // 128,
                NMT=N // 128, F=4 * E, FT=4 * E // 128)


def _build(scale=1, loopn=1, sim_safe=False):
    import os
    import concourse.bacc as bacc
    import concourse.mybir as mybir
    from concourse import tile

    ablate = os.environ.get("ABLATE", "")

    dm = _dims(scale)
    N, M, E, R = dm["N"], dm["M"], dm["E"], dm["R"]
    NT, ET, MT, NMT, F, FT = dm["NT"], dm["ET"], dm["MT"], dm["NMT"], dm["F"], dm["FT"]

    F32 = mybir.dt.float32
    F32R = mybir.dt.float32r
    B16 = mybir.dt.bfloat16
    AF = mybir.ActivationFunctionType
    ALU = mybir.AluOpType
    AX = mybir.AxisListType

    nc = bacc.Bacc("TRN2", target_bir_lowering=False, debug=False, num_devices=NCORES)

    # DRAM inputs. All "[128, X]" tensors are host-packed so partition p's
    # data is contiguous (row p holds m-tile-strided rows of the source).
    d_xe = nc.dram_tensor("xe", [128, MT * E], B16, kind="ExternalInput")
    d_hh = nc.dram_tensor("hhpack", [128, NMT * (E + R)], B16, kind="ExternalInput")
    d_w = {b: nc.dram_tensor(f"wst_{b}", [128, MT * R], B16, kind="ExternalInput")
           for b in ("ee", "eh", "he")}
    d_xnt = nc.dram_tensor("xnt", [128, ET * R], B16, kind="ExternalInput")
    PROJ = ["q_hh", "k_hh", "v_hh", "q_ee", "k_ee", "v_ee",
            "q_eh", "k_eh", "v_eh", "q_he", "k_he", "v_he"]
    d_wp = {w: nc.dram_tensor(f"w_{w}", [128, ET * E], B16, kind="ExternalInput")
            for w in PROJ}
    d_wf1 = nc.dram_tensor("wf1", [128, FT * ET * 128], B16, kind="ExternalInput")
    d_wf2 = nc.dram_tensor("wf2", [128, FT * E], B16, kind="ExternalInput")
    d_b1t = nc.dram_tensor("b1t", [128, FT], F32, kind="ExternalInput")
    d_id = nc.dram_tensor("ident", [128, 128], B16, kind="ExternalInput")
    d_ones = nc.dram_tensor("onesrow", [1, 128], B16, kind="ExternalInput")
    d_b2 = nc.dram_tensor("bias2", [1, E], B16, kind="ExternalInput")
    d_out = nc.dram_tensor("out", [R, E], F32, kind="ExternalOutput")

    with tile.TileContext(nc) as tc:
        with (
            tc.tile_pool(name="xeres", bufs=1) as xr,
            tc.tile_pool(name="stream", bufs=4) as st,
            tc.tile_pool(name="wproj", bufs=1) as ws,
            tc.tile_pool(name="tstore", bufs=1) as ts_,
            tc.tile_pool(name="xnts", bufs=1) as xs,
            tc.tile_pool(name="qkv", bufs=1) as qs,
            tc.tile_pool(name="sdpa", bufs=1) as sp,
            tc.tile_pool(name="xacc", bufs=1) as xa,
            tc.tile_pool(name="ffn", bufs=2) as fs,
            tc.tile_pool(name="zst", bufs=1) as zs,
            tc.tile_pool(name="outp", bufs=1) as op_,
            tc.tile_pool(name="psum", bufs=1, space="PSUM") as pp,
            tc.tile_pool(name="misc", bufs=1) as mp,
        ):
            def body(iv=None):
                qkv = {}

                # ---- small residents
                xnt = xs.tile([128, ET * R], B16, tag="xnt", name="xnt")
                nc.sync.dma_start(out=xnt[:], in_=d_xnt.ap())
                ident = mp.tile([128, 128], B16, tag="ident")
                nc.sync.dma_start(out=ident[:], in_=d_id.ap())
                eps_t = mp.tile([128, 1], F32, tag="eps")
                nc.gpsimd.memset(eps_t[:], 1e-6)
                b1 = mp.tile([128, FT], F32, tag="b1")
                nc.sync.dma_start(out=b1[:], in_=d_b1t.ap())
                ones_t = mp.tile([1, 128], B16, tag="ones")
                nc.sync.dma_start(out=ones_t[:], in_=d_ones.ap())
                b2_t = mp.tile([1, E], B16, tag="b2")
                nc.sync.dma_start(out=b2_t[:], in_=d_b2.ap())

                xe_res = xr.tile([128, MT * E], B16, tag="xeres", name="xeres")
                tstore = {}

                # ---- grouped projections
                _wslot = [0]

                def load_w(name):
                    s = _wslot[0] = (_wslot[0] + 1) % 2
                    wt = ws.tile([128, ET * E], B16, tag=f"wp{s}")
                    nc.sync.dma_start(out=wt[:], in_=d_wp[name].ap())
                    return wt

                def proj_group(specs, banks):
                    """specs: list of (name, src_fn). src_fn(k, t) -> lhsT AP
                    [128, 128] (contraction k-slice, output t-slice)."""
                    wts = {name: load_w(name) for name, _ in specs}
                    for t in range(NT):
                        psb = {}
                        for i, (name, srcs) in enumerate(specs):
                            psb[name] = pp.tile([128, E], F32, tag=f"pbank{banks[i]}",
                                                name=f"ps_{name}{t}")
                        for name, srcs in specs:
                            for k in range(ET):
                                nc.tensor.matmul(
                                    psb[name][:], srcs(k, t),
                                    wts[name][:, k * E:(k + 1) * E],
                                    start=(k == 0), stop=(k == ET - 1))
                        for name, srcs in specs:
                            q = qs.tile([128, E], B16, tag=f"{name[0]}{t}")
                            nc.scalar.copy(q[:], psb[name][:])
                            qkv[(name, t)] = q

                def tsrc(k, t):
                    return tstore[k][:, t * 128:(t + 1) * 128]

                def xsrc(k, t):
                    return xnt[:, k * R + t * 128:k * R + (t + 1) * 128]

                # ---- SDPA (DVE + Pool + ACT)
                x_tiles = [xa.tile([128, E], B16, tag=f"x{t}", name=f"x{t}")
                           for t in range(NT)]
                if ablate == "nosdpa":
                    for t in range(NT):
                        nc.gpsimd.memset(x_tiles[t][:], 0.25)

                def sdpa(branch, t, first):
                    if ablate == "nosdpa":
                        return
                    tree_eng = nc.gpsimd if ablate == "pool" else nc.vector
                    qb = qkv[(f"q_{branch}", t)]
                    kb = qkv[(f"k_{branch}", t)]
                    vb = qkv[(f"v_{branch}", t)]
                    P = sp.tile([128, H * H * D], B16, tag="P")
                    q_ap = qb[:].rearrange("p (h d) -> p h d", h=H).unsqueeze(2) \
                        .broadcast_to((128, H, H, D))
                    k_ap = kb[:].rearrange("p (g d) -> p g d", g=H).unsqueeze(1) \
                        .broadcast_to((128, H, H, D))
                    nc.vector.tensor_tensor(
                        out=P[:].rearrange("p (h g d) -> p h g d", h=H, g=H),
                        in0=q_ap, in1=k_ap, op=ALU.mult)
                    P3 = P[:].rearrange("p (g d) -> p g d", d=D)
                    s2 = sp.tile([128, H * H * D // 2], B16, tag="vt1")
                    s2_3 = s2[:].rearrange("p (g d) -> p g d", d=D // 2)
                    nc.vector.tensor_tensor(
                        out=s2_3, in0=P3[:, :, 0:D // 2], in1=P3[:, :, D // 2:D],
                        op=ALU.add)
                    s_f = sp.tile([128, H * H], F32, tag="s")
                    nc.vector.reduce_sum(out=s_f[:], in_=s2_3, axis=AX.X)
                    Eb = sp.tile([128, H * H], B16, tag="Eb")
                    nc.scalar.activation(Eb[:], s_f[:], AF.Exp)
                    den = sp.tile([128, H], F32, tag="den")
                    nc.vector.reduce_sum(
                        out=den[:], in_=Eb[:].rearrange("p (h g) -> p h g", g=H),
                        axis=AX.X)
                    rec = sp.tile([128, H], F32, tag="rec")
                    nc.vector.reciprocal(rec[:], den[:])
                    Ebn = sp.tile([128, H * H], B16, tag="Ebn")
                    nc.vector.tensor_tensor(
                        out=Ebn[:].rearrange("p (h g) -> p h g", h=H),
                        in0=Eb[:].rearrange("p (h g) -> p h g", h=H),
                        in1=rec[:].unsqueeze(2).broadcast_to((128, H, H)),
                        op=ALU.mult)
                    Pa = sp.tile([128, H * D * H], B16, tag="P")
                    nc.vector.tensor_tensor(
                        out=Pa[:].rearrange("p (h d g) -> p h d g", h=H, d=D),
                        in0=Ebn[:].rearrange("p (h g) -> p h g", h=H).unsqueeze(2)
                            .broadcast_to((128, H, D, H)),
                        in1=vb[:].rearrange("p (d g) -> p d g", g=H).unsqueeze(1)
                            .broadcast_to((128, H, D, H)),
                        op=ALU.mult)
                    # value-path reduction over g (window 8) as a 3-stage
                    # strided add tree on the Pool engine (scalar_tensor_tensor
                    # with op0=bypass hits the generic-op efficiency class)
                    M8 = H * D  # 512 (h,d) groups
                    Pa3 = Pa[:].rearrange("p (m g) -> p m g", g=H)
                    v1 = sp.tile([128, M8 * 4], B16, tag="vt1")
                    v1_3 = v1[:].rearrange("p (m f) -> p m f", f=4)
                    tree_eng.tensor_tensor(
                        out=v1_3, in0=Pa3[:, :, 0:4], in1=Pa3[:, :, 4:8], op=ALU.add)
                    v2 = sp.tile([128, M8 * 2], B16, tag="vt2")
                    v2_3 = v2[:].rearrange("p (m f) -> p m f", f=2)
                    tree_eng.tensor_tensor(
                        out=v2_3, in0=v1_3[:, :, 0:2], in1=v1_3[:, :, 2:4], op=ALU.add)
                    if first:
                        tgt = x_tiles[t]
                    else:
                        tgt = sp.tile([128, E], B16, tag="av")
                    tree_eng.tensor_tensor(
                        out=tgt[:].rearrange("p (m f) -> p m f", f=1),
                        in0=v2_3[:, :, 0:1], in1=v2_3[:, :, 1:2], op=ALU.add)
                    if not first:
                        nc.vector.tensor_tensor(out=x_tiles[t][:], in0=x_tiles[t][:],
                                                in1=tgt[:], op=ALU.add)

                # ---- hh: early node-side projections fill PE during the
                # first stream DMAs, then the packed [xnb | adjt] pass.
                proj_group([("k_hh", xsrc), ("v_hh", xsrc)], banks=(4, 5))
                PB = E + R  # packed row width
                ps_hh = [pp.tile([128, R], F32, tag=f"pbank{e}", name=f"pshh{e}")
                         for e in range(ET)]
                HH_SUB = 2
                for j in range(NMT // HH_SUB):
                    ht = st.tile([128, HH_SUB * PB], B16, tag="stream")
                    nc.sync.dma_start(
                        out=ht[:], in_=d_hh.ap()[:, j * HH_SUB * PB:(j + 1) * HH_SUB * PB])
                    for e in range(ET):
                        for s in range(HH_SUB):
                            m = j * HH_SUB + s
                            nc.tensor.matmul(
                                ps_hh[e][:],
                                ht[:, s * PB + e * 128:s * PB + (e + 1) * 128],
                                ht[:, s * PB + E:s * PB + PB],
                                start=(m == 0), stop=(m == NMT - 1))
                for e in range(ET):
                    tt = ts_.tile([128, R], B16, tag=f"T{e}")
                    nc.scalar.copy(tt[:], ps_hh[e][:])
                    tstore[e] = tt
                proj_group([("q_hh", tsrc)], banks=(2,))
                for t in range(NT):
                    sdpa("hh", t, first=True)

                # ================= generic streaming pass =================
                XE_C = 4096  # xe columns (elements) per interleaved DMA (1 MiB)

                def xe_dma(b):
                    nc.sync.dma_start(
                        out=xe_res[:, b * XE_C:(b + 1) * XE_C],
                        in_=d_xe.ap()[:, b * XE_C:(b + 1) * XE_C])

                def run_pass(branch, pbanks, xe_load=False):
                    W_SUB = 4  # m-tiles per DMA (0.5 MiB)
                    ps = [pp.tile([128, R], F32, tag=f"pbank{pbanks[e]}",
                                  name=f"ps{branch}{e}") for e in range(ET)]
                    if xe_load:
                        xe_dma(0)
                        xe_dma(1)
                    for j in range(MT // W_SUB):
                        # interleave the resident-xe load two blocks ahead of
                        # its first consumer m-tile
                        if xe_load and j % 2 == 0 and j // 2 + 2 < (MT * E) // XE_C:
                            xe_dma(j // 2 + 2)
                        wt = st.tile([128, W_SUB * R], B16, tag="stream")
                        nc.sync.dma_start(
                            out=wt[:],
                            in_=d_w[branch].ap()[:, j * W_SUB * R:(j + 1) * W_SUB * R])
                        for e in range(ET):
                            for s in range(W_SUB):
                                m = j * W_SUB + s
                                nc.tensor.matmul(
                                    ps[e][:],
                                    xe_res[:, m * E + e * 128:m * E + (e + 1) * 128],
                                    wt[:, s * R:(s + 1) * R],
                                    start=(m == 0), stop=(m == MT - 1))
                    for e in range(ET):
                        tt = ts_.tile([128, R], B16, tag=f"T{e}")
                        nc.scalar.copy(tt[:], ps[e][:])
                        tstore[e] = tt

                # ---- ee
                run_pass("ee", (4, 5, 6, 7), xe_load=True)
                proj_group([("q_ee", tsrc), ("k_ee", tsrc)], banks=(4, 5))
                proj_group([("v_ee", xsrc)], banks=(6,))
                for t in range(NT):
                    sdpa("ee", t, first=False)

                # ---- eh
                run_pass("eh", (0, 1, 2, 3))
                proj_group([("q_eh", tsrc)], banks=(0,))
                proj_group([("k_eh", xsrc), ("v_eh", xsrc)], banks=(1, 2))
                for t in range(NT):
                    sdpa("eh", t, first=False)

                # ---- he
                proj_group([("q_he", xsrc), ("v_he", xsrc)], banks=(0, 1))
                run_pass("he", (4, 5, 6, 7))
                proj_group([("k_he", tsrc)], banks=(4,))

                # ================= FFN (two node-halves so half A overlaps
                # the tail SDPA of tiles 2,3) =================
                yT = xa.tile([128, ET * R], B16, tag="yT", name="yT")

                def rms_transpose(t):
                    xt = x_tiles[t]
                    scr = sp.tile([128, E], B16, tag="ffnscr")
                    nc.scalar.activation(scr[:], xt[:], AF.Square)
                    ms = mp.tile([128, 1], F32, tag=f"ms{t}")
                    nc.vector.reduce_sum(out=ms[:], in_=scr[:], axis=AX.X)
                    sd = mp.tile([128, 1], F32, tag=f"sd{t}")
                    nc.scalar.activation(sd[:], ms[:], AF.Sqrt, scale=1.0 / E,
                                         bias=eps_t[:])
                    inv2 = mp.tile([128, 1], F32, tag=f"inv{t}")
                    nc.vector.reciprocal(inv2[:], sd[:])
                    yt = sp.tile([128, E], B16, tag="ffny")
                    nc.vector.tensor_scalar_mul(yt[:], xt[:], inv2[:])
                    for e in range(ET):
                        pst = pp.tile([128, 128], B16, tag=f"pbank{e % 2}")
                        nc.tensor.transpose(pst[:], yt[:, e * 128:(e + 1) * 128],
                                            ident[:])
                        nc.scalar.copy(yT[:, e * R + t * 128:e * R + (t + 1) * 128],
                                       pst[:])

                def ffn_half(h):
                    RH = R // 2  # node columns per half
                    c0 = h * RH
                    pso = [pp.tile([128, E], F32, tag=f"pbank{5 + t}",
                                   name=f"pso{h}{t}") for t in range(2)]
                    for f in range(FT):
                        w1 = fs.tile([128, ET * 128], B16, tag="wf1s")
                        nc.sync.dma_start(
                            out=w1[:], in_=d_wf1.ap()[:, f * E:(f + 1) * E])
                        w2 = fs.tile([128, E], B16, tag="wf2s")
                        nc.sync.dma_start(
                            out=w2[:], in_=d_wf2.ap()[:, f * E:(f + 1) * E])
                        psz = pp.tile([128, RH], F32, tag=f"pbank{2 + (f % 2)}")
                        for k in range(ET):
                            nc.tensor.matmul(
                                psz[:], w1[:, k * 128:(k + 1) * 128],
                                yT[:, k * R + c0:k * R + c0 + RH],
                                start=(k == 0), stop=(k == ET - 1))
                        zt = zs.tile([128, RH], B16, tag=f"zT{f % 2}")
                        nc.scalar.activation(zt[:], psz[:],
                                             AF.Identity if sim_safe else AF.Gelu,
                                             bias=b1[:, f:f + 1])
                        for t in range(2):
                            nc.tensor.matmul(pso[t][:], zt[:, t * 128:(t + 1) * 128],
                                             w2[:],
                                             start=(f == 0), stop=False)
                    for t in range(2):
                        nc.tensor.matmul(pso[t][:], ones_t[0:1, :], b2_t[0:1, :],
                                         start=False, stop=True)
                        ot = op_.tile([128, E], F32, tag="ot")
                        nc.scalar.copy(ot[:], pso[t][:])
                        r0 = (2 * h + t) * 128
                        nc.sync.dma_start(out=d_out.ap()[r0:r0 + 128, :], in_=ot[:])

                sdpa("he", 0, first=False)
                sdpa("he", 1, first=False)
                rms_transpose(0)
                rms_transpose(1)
                sdpa("he", 2, first=False)
                sdpa("he", 3, first=False)
                ffn_half(0)
                rms_transpose(2)
                rms_transpose(3)
                ffn_half(1)

            if loopn > 1:
                with tc.For_i(0, loopn, 1) as _i:
                    body(_i)
            else:
                body()

    nc.compile()
    return nc


def _prep_inputs(inputs, scale=1):
    """Host-side folding + packing + sharding. Returns per-core in_maps."""
    dm = _dims(scale)
    N, M, E, R, F, FT = dm["N"], dm["M"], dm["E"], dm["R"], dm["F"], dm["FT"]
    ET, MT, NMT = dm["ET"], dm["MT"], dm["NMT"]
    x_node = np.asarray(inputs["x_node"], np.float32)
    x_edge = np.asarray(inputs["x_edge"], np.float32)
    adj = np.asarray(inputs["adj"], np.float32)
    g_n = np.asarray(inputs["g_n"], np.float32)
    g_e = np.asarray(inputs["g_e"], np.float32)
    g2 = np.asarray(inputs["g2"], np.float32)

    inv_n = (1.0 / np.sqrt((x_node.astype(np.float64) ** 2).mean(axis=1) + 1e-6)).astype(np.float32)
    inv_e = (1.0 / np.sqrt((x_edge.astype(np.float64) ** 2).mean(axis=1) + 1e-6)).astype(np.float32)
    xn_s = x_node * inv_n[:, None]
    xe_s = x_edge * inv_e[:, None]

    perm = np.array([(j % H) * D + j // H for j in range(E)])  # v cols -> d-major

    def fold_q(w, g):
        return (g[:, None] * np.asarray(w, np.float32)) / np.sqrt(D)

    def fold_k(w, g):
        return g[:, None] * np.asarray(w, np.float32)

    def fold_v(w, g):
        return (g[:, None] * np.asarray(w, np.float32))[:, perm]

    wp = {
        "q_hh": fold_q(inputs["Wq_hh"], g_n),
        "k_hh": fold_k(inputs["Wk_hh"], g_n),
        "v_hh": fold_v(inputs["Wv_hh"], g_n),
        "q_ee": fold_q(inputs["Wq_ee"], g_e),
        "k_ee": fold_k(inputs["Wk_ee"], g_e),
        "v_ee": fold_v(inputs["Wv_ee"], g_n),
        "q_eh": fold_q(inputs["Wq_eh"], g_e),
        "k_eh": fold_k(inputs["Wk_eh"], g_n),
        "v_eh": fold_v(inputs["Wv_eh"], g_n),
        "q_he": fold_q(inputs["Wq_he"], g_n),
        "k_he": fold_k(inputs["Wk_he"], g_e),
        "v_he": fold_v(inputs["Wv_he"], g_n),
    }

    def mpack(a):
        # [rows, C] -> [128, (rows/128) * C]; partition p block j = a[j*128+p]
        rows, C = a.shape
        return np.ascontiguousarray(
            a.reshape(rows // 128, 128, C).transpose(1, 0, 2).reshape(128, -1))

    wf1 = (g2[:, None] * np.asarray(inputs["Wf1"], np.float32))  # [E, F]
    # wf1 pack: partition p, col-block (f*ET+k) = wf1[k*128+p, f*128:(f+1)*128]
    wf1p = wf1.reshape(ET, 128, FT, 128).transpose(1, 2, 0, 3).reshape(128, -1)
    wf2 = np.asarray(inputs["Wf2"], np.float32)  # [F, E]
    wf2p = mpack(wf2)
    bf1 = np.asarray(inputs["bf1"], np.float32)
    b1t = np.ascontiguousarray(bf1.reshape(FT, 128).T)

    shared = {
        "xe": mpack(xe_s).astype(BF16),
        "b1t": b1t,
        "wf1": np.ascontiguousarray(wf1p).astype(BF16),
        "wf2": wf2p.astype(BF16),
        "ident": np.eye(128, dtype=np.float32).astype(BF16),
        "onesrow": np.ones((1, 128), BF16),
        "bias2": np.asarray(inputs["bf2"], np.float32)[None, :].astype(BF16),
    }
    for k, v in wp.items():
        # [E, E] -> [128, ET*E]: partition p block k = W[k*128+p, :]
        shared[f"w_{k}"] = mpack(v).astype(BF16)

    wp1 = {b: np.asarray(inputs[f"Wp1_{b}"], np.float32) for b in ("ee", "eh", "he")}
    xnb = xn_s.astype(BF16)
    in_maps = []
    for c in range(NCORES):
        rows = slice(c * R, (c + 1) * R)
        m = dict(shared)
        # hhpack: per m-tile j: [xnb rows | adjt rows] where adjt = adj[rows].T
        adjt = np.ascontiguousarray(adj[rows].T).astype(BF16)  # [N, R]
        hh = np.concatenate([xnb.reshape(NMT, 128, E),
                             adjt.reshape(NMT, 128, R)], axis=2)  # [NMT,128,E+R]
        m["hhpack"] = np.ascontiguousarray(hh.transpose(1, 0, 2).reshape(128, -1))
        m["xnt"] = mpack(np.ascontiguousarray(xn_s[rows].T)).astype(BF16)
        for b in ("ee", "eh", "he"):
            m[f"wst_{b}"] = mpack(np.ascontiguousarray(wp1[b][rows].T)).astype(BF16)
        in_maps.append(m)
    return in_maps


def kernel(**inputs) -> np.ndarray:
    from concourse.bass_utils import run_bass_kernel_spmd

    if "nc" not in _CACHE:
        _CACHE["nc"] = _build()
    nc = _CACHE["nc"]
    in_maps = _prep_inputs(inputs)
    res = run_bass_kernel_spmd(nc, in_maps, list(range(NCORES)))
    out = np.concatenate([res.results[c]["out"] for c in range(NCORES)], axis=0)
    return np.ascontiguousarray(out, dtype=np.float32)
